# revision 1
# baseline (speedup 1.0000x reference)
"""Trainium2 Bass kernel for ChannelMaxPool top-k masking.

Reference computation:
  x: (B=32, C=512, H=128, W=128) f32
  scores[b,c] = max |x[b,c,:,:]|
  top-128 channels by score (descending, jax.lax.top_k tie order)
  w[b,k] = exp(s_k - m) / sum_selected exp(s_j - m)
    (the global softmax denominator cancels under renormalization)
  y[b,k,:,:] = x[b, idx_k, :, :] * w[b,k]

Sharding: pure data-parallel, batch split across 8 NeuronCores
(4 samples per core), no communication.

Per-core kernel, per-sample pipelined so each sample's selection
epilogue overlaps the next samples' score-pass DMA loads (the score
pass runs at ~356 GB/s, essentially HBM peak):
  pass 1   stream x as (128ch x 8192) tiles on HWDGE queues, DVE
           absmax-reduce -> per-channel scores; the last sample's
           final tile is split 4-way so its selection chain starts
           earlier
  reshape  SBUF->SBUF DMAs transpose scores to one (1, 512) row
  top-k    16x vector.max / max_index / match_replace (top-8 at a
           time, descending; matches jax.lax.top_k tie semantics --
           ties do occur in this dataset)
  gather   all 4 per-split index vectors computed in 2 fused DVE ops
           so the indirect-DMA gathers issue back-to-back, BEFORE the
           weight computation; only 1/4 of x is re-read
  weights  exp/sum/reciprocal on the top-k values, overlapped with
           the gather DMAs
  scale    copy-with-scale by w (Scalar engine for overlapped
           samples, DVE for the exposed last sample), store in 4
           splits
  defer    the second-to-last sample's stores are emitted last on the
           sync stream: they execute inside the DMA-idle window while
           the last sample's top-k chain runs, instead of competing
           with its loads

Measured: ~592 us/core on trn2 (192 MiB traffic/core; HBM roofline
~562 us @ 358 GB/s).
"""

import numpy as np

B, C, H, W = 32, 512, 128, 128
S = H * W
K = 128
N_CORES = 8
BL = B // N_CORES

S_TILE = 8192
G_SPLITS = 4
NEG_INF = -1e30


def _build_nc():
    import concourse.bass as bass
    import concourse.mybir as mybir
    from concourse import bacc
    from concourse.tile import TileContext

    f32 = mybir.dt.float32
    u32 = mybir.dt.uint32
    i32 = mybir.dt.int32

    CCH = C // 128
    NT = S // S_TILE
    GW = S // G_SPLITS

    nc = bacc.Bacc()
    x = nc.dram_tensor("x", [BL, C, S], f32, kind="ExternalInput")
    y = nc.dram_tensor("y", [BL, K, S], f32, kind="ExternalOutput")

    x_flat = x[:].rearrange("b c (g s) -> (b c g) s", g=G_SPLITS)

    with TileContext(nc) as tc:
        with (
            tc.tile_pool(name="load", bufs=2) as load_pool,
            tc.tile_pool(name="gather", bufs=3) as gather_pool,
            tc.tile_pool(name="defer", bufs=4) as defer_pool,
            tc.tile_pool(name="small", bufs=2) as small,
        ):
            # constant per-split offsets [0..G_SPLITS) as f32, one row per k
            iota_h_i = small.tile([K, G_SPLITS], i32, tag="iota_h_i")
            nc.gpsimd.iota(
                iota_h_i[:], pattern=[[1, G_SPLITS]], base=0, channel_multiplier=0
            )
            iota_h = small.tile([K, G_SPLITS], f32, tag="iota_h")
            nc.vector.tensor_copy(iota_h[:], iota_h_i[:])

            deferred_stores = []
            for b in range(BL):
                # ---- pass 1: per-channel absolute max for this sample ----
                FINE = 4  # sub-splits of the last tile of the last sample
                n_par = CCH * NT + (FINE - 1 if b == BL - 1 else 0)
                partials = small.tile([128, CCH * NT + FINE - 1], f32, tag="partials")
                for ci in range(CCH):
                    for t in range(NT):
                        last_tile = b == BL - 1 and ci == CCH - 1 and t == NT - 1
                        sub = FINE if last_tile else 1
                        sw = S_TILE // sub
                        for u in range(sub):
                            tile_in = load_pool.tile([128, S_TILE], f32, tag="ld")
                            s0 = t * S_TILE + u * sw
                            nc.sync.dma_start(
                                out=tile_in[:, :sw],
                                in_=x[b, ci * 128 : (ci + 1) * 128, s0 : s0 + sw],
                            )
                            col = ci * NT + t + u
                            nc.vector.tensor_reduce(
                                out=partials[:, col : col + 1],
                                in_=tile_in[:, :sw],
                                axis=mybir.AxisListType.X,
                                op=mybir.AluOpType.max,
                                apply_absolute_value=True,
                            )
                scores_col = small.tile([128, CCH], f32, tag="scores_col")
                if b < BL - 1:
                    nc.vector.tensor_reduce(
                        out=scores_col[:],
                        in_=partials[:, : CCH * NT].rearrange("p (g t) -> p g t", t=NT),
                        axis=mybir.AxisListType.X,
                        op=mybir.AluOpType.max,
                    )
                else:
                    nc.vector.tensor_reduce(
                        out=scores_col[:, : CCH - 1],
                        in_=partials[:, : (CCH - 1) * NT].rearrange(
                            "p (g t) -> p g t", t=NT
                        ),
                        axis=mybir.AxisListType.X,
                        op=mybir.AluOpType.max,
                    )
                    nc.vector.tensor_reduce(
                        out=scores_col[:, CCH - 1 : CCH],
                        in_=partials[:, None, (CCH - 1) * NT : n_par],
                        axis=mybir.AxisListType.X,
                        op=mybir.AluOpType.max,
                    )
                # ---- transpose scores to one row via SBUF->SBUF DMAs ----
                scores_row = small.tile([1, C], f32, tag="scores_row")
                for ci in range(CCH):
                    nc.sync.dma_start(
                        out=scores_row[:, ci * 128 : (ci + 1) * 128],
                        in_=scores_col[:, ci : ci + 1],
                    )
                # ---- top-K via repeated top-8 extraction (descending),
                #      consuming scores_row in place ----
                topk_vals = small.tile([1, K], f32, tag="topk_vals")
                topk_idx = small.tile([1, K], u32, tag="topk_idx")
                for i in range(K // 8):
                    sl = slice(i * 8, (i + 1) * 8)
                    nc.vector.max(out=topk_vals[:, sl], in_=scores_row[:])
                    nc.vector.max_index(
                        out=topk_idx[:, sl],
                        in_max=topk_vals[:, sl],
                        in_values=scores_row[:],
                    )
                    if i < K // 8 - 1:
                        nc.vector.match_replace(
                            out=scores_row[:],
                            in_to_replace=topk_vals[:, sl],
                            in_values=scores_row[:],
                            imm_value=NEG_INF,
                        )
                # ---- indices first: transpose row->col, start gathers ----
                idx_col_u = small.tile([K, 1], u32, tag="idx_col_u")
                nc.sync.dma_start(out=idx_col_u[:], in_=topk_idx[:])
                idx_col_f = small.tile([K, 1], f32, tag="idx_col_f")
                nc.vector.tensor_copy(idx_col_f[:], idx_col_u[:])
                idx4_f = small.tile([K, G_SPLITS], f32, tag="idx4_f")
                nc.vector.scalar_tensor_tensor(
                    out=idx4_f[:],
                    in0=idx_col_f[:].to_broadcast([K, G_SPLITS]),
                    scalar=float(G_SPLITS),
                    in1=iota_h[:],
                    op0=mybir.AluOpType.mult,
                    op1=mybir.AluOpType.add,
                )
                idx4_i = small.tile([K, G_SPLITS], i32, tag="idx4_i")
                nc.vector.tensor_scalar(
                    out=idx4_i[:],
                    in0=idx4_f[:],
                    scalar1=float(b * C * G_SPLITS),
                    scalar2=None,
                    op0=mybir.AluOpType.add,
                )
                g_tiles = []
                pool_b = defer_pool if b == BL - 2 else gather_pool
                tag_b = "gd" if b == BL - 2 else "g"
                for h in range(G_SPLITS):
                    g = pool_b.tile([K, GW], f32, tag=tag_b)
                    nc.gpsimd.indirect_dma_start(
                        out=g[:],
                        out_offset=None,
                        in_=x_flat,
                        in_offset=bass.IndirectOffsetOnAxis(
                            ap=idx4_i[:, h : h + 1], axis=0
                        ),
                    )
                    g_tiles.append(g)
                # ---- weights (overlap the gather DMAs) ----
                negm = small.tile([1, 1], f32, tag="negm")
                nc.scalar.mul(out=negm[:], in_=topk_vals[:, 0:1], mul=-1.0)
                e = small.tile([1, K], f32, tag="e")
                nc.scalar.activation(
                    out=e[:],
                    in_=topk_vals[:],
                    func=mybir.ActivationFunctionType.Exp,
                    bias=negm[:, 0:1],
                    scale=1.0,
                )
                ssum = small.tile([1, 1], f32, tag="ssum")
                nc.vector.reduce_sum(out=ssum[:], in_=e[:], axis=mybir.AxisListType.X)
                sinv = small.tile([1, 1], f32, tag="sinv")
                nc.vector.reciprocal(out=sinv[:], in_=ssum[:])
                w_row = small.tile([1, K], f32, tag="w_row")
                nc.vector.tensor_scalar_mul(w_row[:], e[:], sinv[:, 0:1])
                w_col = small.tile([K, 1], f32, tag="w_col")
                nc.sync.dma_start(out=w_col[:], in_=w_row[:])
                # ---- scale + store (sample BL-2's stores are deferred) ----
                for h in range(G_SPLITS):
                    g = g_tiles[h]
                    if b < BL - 1:
                        nc.scalar.activation(
                            out=g[:],
                            in_=g[:],
                            func=mybir.ActivationFunctionType.Copy,
                            bias=0.0,
                            scale=w_col[:, 0:1],
                        )
                    else:
                        nc.vector.tensor_scalar_mul(g[:], g[:], w_col[:, 0:1])
                    if b == BL - 2:
                        deferred_stores.append((b, h, g))
                    else:
                        nc.sync.dma_start(
                            out=y[b, :, h * GW : (h + 1) * GW], in_=g[:]
                        )
                if b == BL - 1:
                    # emitted last on the sync stream: these fill the DMA-idle
                    # window while the last sample's top-k chain runs
                    for db, dh, dg in deferred_stores:
                        nc.sync.dma_start(
                            out=y[db, :, dh * GW : (dh + 1) * GW], in_=dg[:]
                        )
    if not nc.is_finalized():
        nc.finalize()
    return nc


_NC_CACHE = None


def _get_nc():
    global _NC_CACHE
    if _NC_CACHE is None:
        _NC_CACHE = _build_nc()
    return _NC_CACHE


def _run(x, trace=False):
    from concourse.bass_utils import run_bass_kernel_spmd

    nc = _get_nc()
    xr = np.ascontiguousarray(x, dtype=np.float32).reshape(N_CORES, BL, C, S)
    in_maps = [{"x": xr[c]} for c in range(N_CORES)]
    res = run_bass_kernel_spmd(nc, in_maps, list(range(N_CORES)), trace=trace)
    out = np.empty((B, K, H, W), dtype=np.float32)
    for c in range(N_CORES):
        out[c * BL : (c + 1) * BL] = res.results[c]["y"].reshape(BL, K, H, W)
    return out, res


def kernel(x):
    out, _ = _run(x, trace=False)
    return out

